# revision 1
# baseline (speedup 1.0000x reference)
"""Trainium2 Bass kernel for LinearAttention4 (self-contained).

Problem (per sample): x [256, 56, 56] fp32
  qk = elu(conv1x1(x; qk_w, qk_b)) + 1 ; q, k = split(qk)
  kv = k @ v.T / n ; num = q.T @ kv ; den = q.T @ mean(k) + 1e-6
  attn = (num / den).T ; out = attn + depthwise3x3(x; pe_w) + pe_b

Sharding: data-parallel over batch, 4 samples per core on 8 NeuronCores.
All matmuls run as float32r (TF32-like, 1 cyc/row at N>=256).

Everything on-chip lives in PADDED spatial coordinates (58x58 zero-padded
grid, flattened to 3364 per channel block) so that every matmul operand is
a contiguous 1-free-dim AP (HW requirement). The 3x3 depthwise conv taps
are then pure offsets +-{58,1} into the padded buffer; pad columns yield
garbage outputs which the PSUM->SBUF evacuation AP skips. The kv
contraction runs over all 3364 padded positions: x's pad positions are
zero so they contribute nothing (k's pad columns are explicitly zeroed).

Per-core pipeline (per sample):
  A) DMA host-padded x [2, 128, 3364] -> SBUF
  B) qk matmul (lhsT = qk_w.T chunks, rhs = x spans of 464) + elu+1 via
     min(exp(z+b),1) + relu(z+b)  [exact identity for elu(z+b)+1]
  C) PE-transpose x and k in 116-wide chunks; kv = kT.T @ xT (+ k_sum via
     an N=1 matmul against ones into col 256 of the same psum tile)
  D) den = k_sum.T @ q per span; fold [1,3364]->[116,29] via DMA, +eps,
     reciprocal, unfold, gpsimd partition_broadcast; q *= recip
     (exact: the den scale commutes past the kv contraction)
  E) num matmul + 9 diagonal conv-tap matmuls accumulate into ONE psum
     tile per (c-block, span); ACT evacuates psum + pe_b -> out,
     compacting padded coords back to dense 56x56
"""

import numpy as np

import concourse.bass as bass
import concourse.mybir as mybir
from concourse.tile import TileContext
from concourse.bass_utils import run_bass_kernel_spmd

F32 = mybir.dt.float32
F32R = mybir.dt.float32r

B, C, H, W = 32, 256, 56, 56
N = H * W  # 3136
NCORES = 8
SPC = B // NCORES  # 4
HP = H + 2  # 58
NP = HP * HP  # 3364
SPAN = 8 * HP  # 464 cols per qk/num/conv chunk (8 padded rows)
NCH = 7  # chunks of 8 interior rows
TCH = 116  # transpose chunk width (3364 = 29 * 116)
NTC = NP // TCH  # 29
EPS = 1e-6 * N  # den eps, rescaled because kv/k_sum stay unscaled


def _split_multi_waits(nc, max_waits=1):
    """Walrus here allows one SyncWait per instruction; hoist extras onto
    fresh same-engine NOPs placed immediately before (same semantics)."""
    for f in nc.m.functions:
        for blk in f.blocks:
            new_insts = []
            for ins in blk.instructions:
                si = ins.sync_info
                waits = list(si.on_wait) if si is not None else []
                if len(waits) > max_waits:
                    head, tail = waits[:-max_waits], waits[-max_waits:]
                    for w in head:
                        nop = mybir.InstNoOp(
                            name=f"Wsplit-{nc.next_id()}", engine=ins.engine,
                            ins=[], outs=[],
                        )
                        nop.sync_info = mybir.SyncInfo(on_wait=[w], on_update=[])
                        new_insts.append(nop)
                    ins.sync_info = mybir.SyncInfo(
                        on_wait=tail, on_update=list(si.on_update)
                    )
                new_insts.append(ins)
            blk.instructions = new_insts


def _build():
    nc = bass.Bass()
    # all DRAM params are flat 1D: PJRT/XLA may permute multi-dim parameter
    # layouts (observed: [2,128,NP] stored as [128,2,NP]); 1D is unambiguous
    xs_f = nc.declare_dram_parameter("xs", [SPC * 2 * 128 * NP], F32R, isOutput=False)
    wqkT_f = nc.declare_dram_parameter("wqkT", [2 * 128 * 256], F32R, isOutput=False)
    wtap_f = nc.declare_dram_parameter("wtap", [2 * 9 * 128 * 128], F32R, isOutput=False)
    ident_f = nc.declare_dram_parameter("ident", [128 * 128], F32R, isOutput=False)
    ones_f = nc.declare_dram_parameter("ones", [128 * 128], F32R, isOutput=False)
    biasqk_f = nc.declare_dram_parameter("biasqk", [128 * 2], F32, isOutput=False)
    peb_f = nc.declare_dram_parameter("peb", [128 * 2], F32, isOutput=False)
    out_f = nc.declare_dram_parameter("out", [SPC * 2 * 128 * N], F32, isOutput=True)
    xs = xs_f[:].rearrange("(s c p n) -> s c p n", s=SPC, c=2, p=128)
    out = out_f[:].rearrange("(s c p n) -> s c p n", s=SPC, c=2, p=128)

    Exp = mybir.ActivationFunctionType.Exp
    Relu = mybir.ActivationFunctionType.Relu
    Ident = mybir.ActivationFunctionType.Identity
    mi, ad, mx = mybir.AluOpType.min, mybir.AluOpType.add, mybir.AluOpType.max

    def span_start(ch):
        # first output position of chunk ch, in padded coords
        return HP * (1 + 8 * ch) + 1

    with TileContext(nc) as tc:
        with (
            tc.tile_pool(name="wp", bufs=1) as wp,
            tc.tile_pool(name="xpool", bufs=2) as xpool,
            tc.tile_pool(name="qkpool", bufs=2) as qkpool,
            tc.tile_pool(name="erpool", bufs=2) as erpool,
            tc.tile_pool(name="xkpool", bufs=3) as xkpool,
            tc.tile_pool(name="kvpool", bufs=2) as kvpool,
            tc.tile_pool(name="denpool", bufs=1) as denpool,
            tc.tile_pool(name="opool", bufs=2) as opool,
            tc.tile_pool(name="bigps", bufs=3, space="PSUM") as bigps,
            tc.tile_pool(name="trps", bufs=2, space="PSUM") as trps,
            tc.tile_pool(name="kvps", bufs=2, space="PSUM") as kvps,
            tc.tile_pool(name="dbps", bufs=1, space="PSUM") as dbps,
        ):
            w_qk = wp.tile([128, 512], F32R, name="w_qk")
            w_tap = wp.tile([128, 2304], F32R, name="w_tap")
            w_id = wp.tile([128, 128], F32R, name="w_id")
            w_ones = wp.tile([128, 128], F32R, name="w_ones")
            w_bqk = wp.tile([128, 2], F32, name="w_bqk")
            w_peb = wp.tile([128, 2], F32, name="w_peb")
            dma = nc.default_dma_engine.dma_start
            dma(
                out=w_qk[:].rearrange("p (c o) -> p c o", c=2),
                in_=wqkT_f[:].rearrange("(c p o) -> p c o", c=2, p=128),
            )
            dma(
                out=w_tap[:].rearrange("p (c t j) -> p c t j", c=2, t=9),
                in_=wtap_f[:].rearrange("(c t p j) -> p c t j", c=2, t=9, p=128),
            )
            dma(out=w_id[:], in_=ident_f[:].rearrange("(p j) -> p j", p=128))
            dma(out=w_ones[:], in_=ones_f[:].rearrange("(p j) -> p j", p=128))
            dma(out=w_bqk[:], in_=biasqk_f[:].rearrange("(p c) -> p c", p=128))
            dma(out=w_peb[:], in_=peb_f[:].rearrange("(p c) -> p c", p=128))

            for s in range(SPC):
                # ---- A: load padded x -------------------------------------
                xp = xpool.tile([128, 2 * NP + 2], F32R, tag="xp", name="xp")
                for cb in range(2):
                    dma(out=xp[:, NP * cb : NP * (cb + 1)], in_=xs[s, cb])

                # ---- B: qk matmul + elu+1 ---------------------------------
                q_elu = qkpool.tile([128, NP], F32R, tag="qelu", name="q_elu")
                k_elu = qkpool.tile([128, NP], F32R, tag="kelu", name="k_elu")
                ksum7 = denpool.tile([128, 8], F32, tag="ksum7", name="ksum7")
                ksum = denpool.tile([128, 2], F32R, tag="ksum", name="ksum")
                # zero k's pad positions at tile birth (elu writes interior
                # only) so the kv/k_sum contraction over all 3364 padded
                # positions matches the dense reference exactly
                k_f32 = k_elu[:].bitcast(F32)
                nc.vector.memset(k_f32[:, 0:59], 0)
                nc.vector.memset(k_f32[:, NP - 58 : NP], 0)
                nc.vector.memset(
                    k_f32.rearrange("p (y x) -> p y x", y=HP)[:, 1:57, 0:1], 0
                )
                nc.vector.memset(
                    k_f32.rearrange("p (y x) -> p y x", y=HP)[:, 1:57, 57:58], 0
                )
                nc.vector.memset(ksum7[:].bitcast(F32), 0)
                for mb in range(2):  # 0 = q, 1 = k
                    dst = q_elu if mb == 0 else k_elu
                    for ch in range(NCH):
                        p1 = span_start(ch)
                        ps = bigps.tile([128, SPAN], F32, tag="bigps", name="ps")
                        for cc in range(2):
                            nc.tensor.matmul(
                                ps[:],
                                w_qk[:, 256 * cc + 128 * mb : 256 * cc + 128 * mb + 128],
                                xp[:, NP * cc + p1 : NP * cc + p1 + SPAN],
                                start=(cc == 0),
                                stop=(cc == 1),
                            )
                        e = erpool.tile([128, SPAN], F32, tag="e", name="e")
                        r = erpool.tile([128, SPAN], F32, tag="r", name="r")
                        nc.scalar.activation(
                            e[:], ps[:], Exp, bias=w_bqk[:, mb : mb + 1], scale=1.0
                        )
                        nc.vector.tensor_scalar(
                            out=r[:], in0=ps[:], scalar1=w_bqk[:, mb : mb + 1],
                            scalar2=0.0, op0=ad, op1=mx,
                        )
                        dst_v = dst[:, p1 : p1 + SPAN].rearrange(
                            "p (a b) -> p a b", b=HP
                        )[:, :, 0:56]
                        e_v = e[:].rearrange("p (a b) -> p a b", b=HP)[:, :, 0:56]
                        r_v = r[:].rearrange("p (a b) -> p a b", b=HP)[:, :, 0:56]
                        nc.vector.scalar_tensor_tensor(
                            dst_v, e_v, 1.0, r_v, op0=mi, op1=ad,
                            accum_out=(
                                ksum7[:, ch : ch + 1] if mb == 1 else None
                            ),
                        )
                with nc.allow_low_precision(
                    reason="ksum reduce to f32r: feeds f32r matmul anyway"
                ):
                    nc.vector.tensor_reduce(
                        ksum[:, 0:1], ksum7[:], op=mybir.AluOpType.add,
                        axis=mybir.AxisListType.X,
                    )

                # ---- C: transposes + kv -----------------------------------
                kvp = kvps.tile([128, 256], F32, tag="kvps", name="kvp")
                kv_sb = kvpool.tile([128, 256], F32R, tag="kv", name="kv_sb")
                for j in range(NTC):
                    tp = trps.tile([TCH, 384], F32R, tag="trps", name="tp")
                    for cb in range(2):
                        nc.tensor.transpose(
                            tp[:, 128 * cb : 128 * (cb + 1)],
                            xp[:, NP * cb + TCH * j : NP * cb + TCH * (j + 1)],
                            w_id[:],
                        )
                    nc.tensor.transpose(
                        tp[:, 256:384],
                        k_elu[:, TCH * j : TCH * (j + 1)],
                        w_id[:],
                    )
                    xk = xkpool.tile([TCH, 384], F32R, tag="xkt", name="xk")
                    if j % 2 == 0:
                        nc.scalar.copy(xk[:], tp[:])
                    else:
                        nc.vector.tensor_copy(xk[:], tp[:])
                    nc.tensor.matmul(
                        kvp[:, 0:256], xk[:, 256:384], xk[:, 0:256],
                        start=(j == 0), stop=(j == NTC - 1),
                    )
                nc.scalar.copy(kv_sb[:], kvp[:])

                # ---- D: den + reciprocal + q scaling ----------------------
                den = denpool.tile([1, NP], F32R, tag="den", name="den")
                for ch in range(NCH):
                    p1 = span_start(ch)
                    dp = dbps.tile([128, SPAN], F32, tag="dbps", name="dp")
                    nc.tensor.matmul(
                        dp[0:1, :], ksum[:, 0:1],
                        q_elu[:, p1 : p1 + SPAN],
                        start=True, stop=True,
                    )
                    nc.scalar.copy(den[:, p1 : p1 + SPAN], dp[0:1, :])
                recf = denpool.tile([TCH, NTC], F32R, tag="recf", name="recf")
                dma(out=recf[:], in_=den[:])
                nc.vector.tensor_scalar_add(recf[:], recf[:], EPS)
                with nc.allow_low_precision(
                    reason="f32r reciprocal: den ~O(n), 6e-5 rel is fine"
                ):
                    nc.vector.reciprocal(recf[:], recf[:])
                rrow = denpool.tile([1, NP], F32R, tag="rrow", name="rrow")
                dma(out=rrow[:], in_=recf[:])
                for ch in range(NCH):
                    p1 = span_start(ch)
                    bc = dbps.tile([128, SPAN], F32, tag="dbps", name="bc")
                    nc.tensor.matmul(
                        bc[:], w_ones[0:1, :], rrow[:, p1 : p1 + SPAN],
                        start=True, stop=True,
                    )
                    nc.vector.tensor_mul(
                        q_elu[:, p1 : p1 + SPAN], q_elu[:, p1 : p1 + SPAN], bc[:]
                    )

                # ---- E: num + conv taps into one psum; evac + bias --------
                for cb in range(2):
                    o_sb = opool.tile([128, N], F32, tag="osb", name="o_sb")
                    for ch in range(NCH):
                        p1 = span_start(ch)
                        pn = bigps.tile([128, SPAN], F32, tag="bigps", name="pn")
                        nc.tensor.matmul(
                            pn[:], kv_sb[:, 128 * cb : 128 * (cb + 1)],
                            q_elu[:, p1 : p1 + SPAN],
                            start=True, stop=False,
                        )
                        for t in range(9):
                            ky, kx = t // 3, t % 3
                            off = HP * (ky - 1) + (kx - 1)
                            nc.tensor.matmul(
                                pn[:],
                                w_tap[:, 1152 * cb + 128 * t : 1152 * cb + 128 * (t + 1)],
                                xp[:, NP * cb + p1 + off : NP * cb + p1 + off + SPAN],
                                start=False, stop=(t == 8),
                            )
                        nc.scalar.activation(
                            o_sb[:, 448 * ch : 448 * (ch + 1)].rearrange(
                                "p (y x) -> p y x", x=56
                            ),
                            pn[:].rearrange("p (y x) -> p y x", x=HP)[:, :, 0:56],
                            Ident, bias=w_peb[:, cb : cb + 1], scale=1.0,
                        )
                    dma(out=out[s, cb], in_=o_sb[:])

    _split_multi_waits(nc)
    return nc


_NC_CACHE = []


def kernel(x, qk_w, qk_b, pe_w, pe_b):
    x = np.asarray(x, np.float32)
    qk_w = np.asarray(qk_w, np.float32)
    qk_b = np.asarray(qk_b, np.float32)
    pe_w = np.asarray(pe_w, np.float32)
    pe_b = np.asarray(pe_b, np.float32)

    # host prep: zero-padded 58x58 spatial layout, c in two partition blocks
    xp = np.zeros((B, 2, 128, HP, HP), np.float32)
    xp[:, :, :, 1 : H + 1, 1 : W + 1] = x.reshape(B, 2, 128, H, W)
    xp = xp.reshape(B, 2, 128, NP)

    wqkT = np.ascontiguousarray(qk_w.T).reshape(2, 128, 256)
    wtap = np.zeros((2, 9, 128, 128), np.float32)
    idx = np.arange(128)
    for cb in range(2):
        for t in range(9):
            wtap[cb, t, idx, idx] = pe_w[128 * cb : 128 * (cb + 1), 0, t // 3, t % 3]
    biasqk = np.stack([qk_b[:128], qk_b[128:]], axis=1).copy()  # [128, 2]
    pebh = np.stack([pe_b[:128], pe_b[128:]], axis=1).copy()

    shared = {
        "wqkT": wqkT.ravel(),
        "wtap": wtap.ravel(),
        "ident": np.eye(128, dtype=np.float32).ravel(),
        "ones": np.ones(128 * 128, np.float32),
        "biasqk": biasqk.ravel(),
        "peb": pebh.ravel(),
    }
    in_maps = [
        {"xs": np.ascontiguousarray(xp[r * SPC : (r + 1) * SPC]).ravel(), **shared}
        for r in range(NCORES)
    ]

    if not _NC_CACHE:
        _NC_CACHE.append(_build())
    nc = _NC_CACHE[0]
    res = run_bass_kernel_spmd(nc, in_maps, list(range(NCORES)))

    full = np.empty((B, C, H, W), np.float32)
    for r in range(NCORES):
        o = res.results[r]["out"].reshape(SPC, 2, 128, N)
        full[r * SPC : (r + 1) * SPC] = o.reshape(SPC, C, H, W)
    return full

